# revision 15
# baseline (speedup 1.0000x reference)
"""MemristorDense forward on 8 Trainium2 NeuronCores.

Math
----
Reference computes, with R = n_in+1 rows (x plus a ones bias-row), C = 2*n_out
interleaved pos/neg columns:

    y = 0.5 * sum_r sign(x) * (W+m9) * exp(L[b,r] * log2(n[r,c]))

with L = ln(max(2|x|,1e-12)), m9 = max_w/9.  Write n = 2*(1+w) (series center
g=1, |w| <~ 0.29) and z = log2(2|x|).  Then sign(x)*exp(L*log2 n) =
2x * (1+w)^z, and the binomial series (1+w)^z = sum_k C(z,k) w^k turns the
[B,R,C] elementwise-pow contraction into K+1 TensorEngine matmuls.  K=2
measures ~3.7e-3 relative against the 2e-2 gate.

Centering at g=1 (instead of the log2-midrange of n) makes the A-side series
start at A0 = x exactly: the (2|x|)^(g-1) Exp correction disappears, so the
device chain is just x^2 -> Ln -> two fused multiplies, and the first matmul's
stationary operand is the raw x tile straight off the DMA.

Re-expanding C(z,k) into powers of z on the host (exact, float64):

    y[:,c] = sum_r x*U0 + (x z)*U1 + (x z^2)*U2
    U0 = T0,  U1 = T1 - T2/2,  U2 = T2/2,   T_k = (wp+m9) wp'^k - (wn+m9) wn'^k

with w' = n/2 - 1 per crossbar column.  The ones bias-row contributes the
b-independent exact vector yb/2 = 0.5*((b_pos+m9) n_pos - (b_neg+m9) n_neg),
added during the host-side unshard.

Sharding: data-parallel over the contraction (input-feature) dim -- core j
takes rows 128j..128j+127, computes a partial [B, n_out] in PSUM, and the
host unshard sums the 8 fp16 partials (f64) and adds yb/2.  This also splits
the serial ScalarE/VectorE x-chain 8 ways instead of replicating it.

Device per core: DMA xT [128,128] + U [128,1536] fp16 (one transfer per
DMA ring, deadline-balanced); chain Ax = x*x (DVE), Lr = Ln(4 Ax + eps)
(ACT), B1 = (c Lr) * x, B2 = (c Lr) * B1 (DVE scalar_tensor_tensor,
c = 1/(2 ln 2), z = c*Lr); 6 matmuls acc_{A,B} += {x,B1,B2}.T @ U_k
(N=256, two PSUM banks so the two evacuation engines run in parallel);
evacuate PSUM->SBUF fp16 on ScalarE (bank A) and DVE (bank B); output
[128,512] fp16 as two halves on the two HWDGE queues.
"""

import numpy as np

import concourse.bacc as bacc
import concourse.tile as tile
import concourse.mybir as mybir
from concourse.bass_utils import run_bass_kernel_spmd

F32 = mybir.dt.float32
F16 = mybir.dt.float16
ALU = mybir.AluOpType
ACT = mybir.ActivationFunctionType

NCORES = 8
B = 128
N_IN = 1024
N_OUT = 512
RS = N_IN // NCORES     # 128 contraction rows per core
LN2 = 0.6931471805599453
C2LN2 = 1.0 / (2.0 * LN2)   # z = C2LN2 * Lr

# Stashed by kernel() for the test harness (exec_time_ns, trace paths).
LAST_RESULTS = None

_ACT_SET = "natural_log_exp_and_others"
_ACT_SHARED = {
    ACT.Square, ACT.Ln, ACT.Exp, ACT.Copy, ACT.Identity, ACT.Abs, ACT.Sign,
    ACT.MemsetZero,
}


def _patched_tables(arch, _orig=bacc.get_activation_tables):
    """Steer the act-table-load pass to a single table set: every function we
    use (ln/copy) lives in natural_log_exp_and_others, but the greedy
    per-instruction chooser would otherwise pick several sets (~1.3us
    ACT_TABLE_LOAD each on the critical ScalarE chain).  Set names and order
    are preserved so act_func_set_id stays a valid act_info.json index."""
    t = _orig(arch)
    return {
        name: (funcs if name == _ACT_SET else (funcs - _ACT_SHARED))
        for name, funcs in t.items()
    }


def _build_program():
    orig_tables = bacc.get_activation_tables
    bacc.get_activation_tables = _patched_tables
    try:
        return _build_program_inner()
    finally:
        bacc.get_activation_tables = orig_tables


def _build_program_inner():
    nc = bacc.Bacc(
        "TRN2", target_bir_lowering=False, debug=False, num_devices=NCORES
    )
    xt_d = nc.dram_tensor("xt_in", [128, RS], F16, kind="ExternalInput").ap()
    wc_d = nc.dram_tensor("wc_in", [128, 3 * N_OUT], F16, kind="ExternalInput").ap()
    y_d = nc.dram_tensor("y_out", [B, N_OUT], F16, kind="ExternalOutput").ap()

    with tile.TileContext(nc) as tc:
        with (
            tc.tile_pool(name="pers", bufs=1) as pool,
            tc.tile_pool(name="acc", bufs=1, space="PSUM") as pspool,
        ):
            eps = pool.tile([128, 1], F32)
            nc.vector.memset(eps[:], 1e-24)
            xT = pool.tile([128, RS], F16)
            Ax = pool.tile([128, RS], F16)
            Lr = pool.tile([128, RS], F16)
            B1 = pool.tile([128, RS], F16)
            B2 = pool.tile([128, RS], F16)
            Wc = pool.tile([128, 3 * N_OUT], F16)
            ysb = pool.tile([128, N_OUT], F16)
            H = N_OUT // 2
            # Two PSUM banks so the two evacuation engines don't get
            # serialized by the same-bank overlap tracker.
            acc_a = pspool.tile([128, H], F32)
            acc_b = pspool.tile([128, H], F32)

            # Exactly one input transfer per DMA ring: a ring's second
            # transfer only lands ~2us after its first completes (measured --
            # queuing W pieces behind x or W0 always lost).  Effective
            # per-ring rate is ~165 GB/s with ~1.4us completion latency, so
            # balance W bytes across the scalar HWDGE ring and the gpsimd
            # SWDGE ring by matmul deadline: scalar gets W0 + W1[bank A]
            # (first needed), gpsimd (whose ~0.9us SWDGE library-load delay
            # makes it land mid-stream) gets W1[bank B] + W2.  x heads the
            # serial chain alone on the sync HWDGE ring.
            SPLIT = N_OUT + N_OUT // 2
            nc.sync.dma_start(xT[:], xt_d[:])
            nc.scalar.dma_start(Wc[:, 0:SPLIT], wc_d[:, 0:SPLIT])
            nc.gpsimd.dma_start(Wc[:, SPLIT:3 * N_OUT], wc_d[:, SPLIT:3 * N_OUT])

            # x chain: Ax = x^2 (DVE); Lr = ln(4 Ax + eps) = 2 ln(2|x|) (ACT,
            # 4x folded into the Ln input scale); z = C2LN2*Lr;
            # B1 = x*z, B2 = B1*z as fused (Lr*c)*t ops on DVE.
            nc.vector.tensor_mul(Ax[:], xT[:], xT[:])
            nc.scalar.activation(Lr[:], Ax[:], ACT.Ln, bias=eps[:], scale=4.0)
            nc.vector.scalar_tensor_tensor(
                B1[:], Lr[:], C2LN2, xT[:], ALU.mult, ALU.mult
            )
            nc.vector.scalar_tensor_tensor(
                B2[:], Lr[:], C2LN2, B1[:], ALU.mult, ALU.mult
            )

            # Two PSUM accumulation groups (bank A = cols 0:H, bank B = rest),
            # N=256 each; stationary = {x, B1, B2}.
            def wsl(k, half):
                lo = k * N_OUT + half * H
                return slice(lo, lo + H)

            nc.tensor.matmul(acc_a[:], xT[:], Wc[:, wsl(0, 0)], start=True, stop=False)
            nc.tensor.matmul(acc_b[:], xT[:], Wc[:, wsl(0, 1)], start=True, stop=False)
            nc.tensor.matmul(acc_a[:], B1[:], Wc[:, wsl(1, 0)], start=False, stop=False)
            nc.tensor.matmul(acc_b[:], B1[:], Wc[:, wsl(1, 1)], start=False, stop=False)
            nc.tensor.matmul(acc_a[:], B2[:], Wc[:, wsl(2, 0)], start=False, stop=True)
            nc.tensor.matmul(acc_b[:], B2[:], Wc[:, wsl(2, 1)], start=False, stop=True)

            # Evacuate PSUM -> SBUF fp16 on two engines in parallel (distinct
            # banks), each half DMA'd out on its own HWDGE queue immediately.
            nc.scalar.copy(ysb[:, 0:H], acc_a[:])
            nc.vector.tensor_copy(ysb[:, H:N_OUT], acc_b[:])
            nc.sync.dma_start(y_d[:, 0:H], ysb[:, 0:H])
            nc.scalar.dma_start(y_d[:, H:N_OUT], ysb[:, H:N_OUT])

    nc.compile()
    return nc


def _shard_inputs(x, w_pos, w_neg, b_pos, b_neg, n_param, m9):
    """Per-core input maps.  Weight transform in float64, rounded to fp16
    once: U0 = T0, U1 = T1 - T2/2, U2 = T2/2 with
    T_k = (wp+m9) wp'^k - (wn+m9) wn'^k, w' = n/2 - 1."""
    Wp = w_pos.astype(np.float64) + m9
    Wn = w_neg.astype(np.float64) + m9
    wp = n_param[:N_IN, 0::2].astype(np.float64) * 0.5 - 1.0
    wn = n_param[:N_IN, 1::2].astype(np.float64) * 0.5 - 1.0
    T1 = Wp * wp - Wn * wn
    T2 = Wp * wp * wp - Wn * wn * wn
    U0 = (Wp - Wn).astype(np.float16)
    U1 = (T1 - 0.5 * T2).astype(np.float16)
    U2 = (0.5 * T2).astype(np.float16)

    xTfull = np.ascontiguousarray(x.astype(np.float16).T)   # [N_IN, B]

    in_maps = []
    for j in range(NCORES):
        rs = slice(RS * j, RS * (j + 1))
        wc = np.concatenate([U0[rs], U1[rs], U2[rs]], axis=1)
        in_maps.append(
            {
                "xt_in": np.ascontiguousarray(xTfull[rs]),
                "wc_in": np.ascontiguousarray(wc),
            }
        )
    return in_maps


def kernel(x, w_pos, w_neg, b_pos, b_neg, n_param, **run_kwargs):
    global LAST_RESULTS
    x = np.ascontiguousarray(np.asarray(x, np.float32))
    w_pos = np.asarray(w_pos, np.float32)
    w_neg = np.asarray(w_neg, np.float32)
    b_pos = np.asarray(b_pos, np.float32)
    b_neg = np.asarray(b_neg, np.float32)
    n_param = np.asarray(n_param, np.float32)

    max_w = float(max(w_pos.max(), w_neg.max(), b_pos.max(), b_neg.max()))
    m9 = max_w / 9.0

    nc = _build_program()
    in_maps = _shard_inputs(x, w_pos, w_neg, b_pos, b_neg, n_param, m9)
    res = run_bass_kernel_spmd(nc, in_maps, list(range(NCORES)), **run_kwargs)
    LAST_RESULTS = res

    # Unshard: sum the 8 contraction partials, add the exact bias-row fold.
    y = np.zeros((B, N_OUT), np.float64)
    for j in range(NCORES):
        y += res.results[j]["y_out"].astype(np.float64)
    nb = n_param[N_IN, :].astype(np.float64)
    yb = (b_pos.astype(np.float64) + m9) * nb[0::2] - (
        b_neg.astype(np.float64) + m9
    ) * nb[1::2]
    y += 0.5 * yb[None, :]
    return y.astype(np.float32)
